# revision 1
# baseline (speedup 1.0000x reference)
"""Masked attention kernel for Trainium2, sharded over 8 NeuronCores.

Problem: B=4, H=16, S=2048, D=64 attention with a boolean mask shared
across heads:  out = softmax((QK^T masked to -1e9) / sqrt(D)) @ V.

Sharding: batch*heads across 8 cores -> each core owns one batch element's
half of the heads (8 heads), so the [S, S] mask is loaded once per core.

Per-core kernel strategy (per pair of heads):
  - Scores are computed TRANSPOSED: ST[k, q] = K @ Q^T, so that the
    post-softmax matrix P^T[k, q] is already laid out with the contraction
    dim (k) on partitions for the second matmul.
  - D=64 contraction lets two heads share the 128x128 PE array via
    row tiling (tile_position (0,0) / (64,0)).
  - exp on the scalar engine directly from PSUM (scale=1/8 folded in),
    output in fp16; mask applied as a multiply by (1-m)^T on the vector
    engine in its 2x 16-bit mode.  exp(-1e9/8) == 0 in fp32, so multiplying
    the exp by (1-m) is exactly equivalent to the reference's additive
    -1e9 mask.  All intermediate P/V tensors are fp16 (same throughput as
    bf16 on every engine, 8x the mantissa).
  - Second matmul uses V augmented with a ones column -> PSUM accumulator
    [65, 512] holds both the output numerator (rows 0..63, transposed) and
    the softmax denominators (row 64).
  - Finalize: PE transpose [65,128] -> [128,65], reciprocal of the sums
    column and a per-partition tensor_scalar multiply, then DMA out.
"""

import numpy as np
import ml_dtypes

B, H, S, D = 4, 16, 2048, 64
N_CORES = 8
HEADS_PER_CORE = (B * H) // N_CORES  # 8

_NC_CACHE = {}


def build_attention_nc(s=S, hpc=HEADS_PER_CORE, qts=512, mm1_dt="bfloat16",
                       reps=1, loop_reps=0):
    """Build the per-core Bass program. Parameterized so a miniature
    version can run under CoreSim."""
    import concourse.bass as bass
    import concourse.mybir as mybir
    import concourse.tile as tile
    from concourse import bacc
    from concourse.masks import make_identity

    bf16 = mybir.dt.bfloat16
    f16 = mybir.dt.float16
    f32 = mybir.dt.float32
    qk_dt = bf16 if mm1_dt == "bf16x2" else getattr(mybir.dt, mm1_dt)
    pv_dt = f16
    Exp = mybir.ActivationFunctionType.Exp

    kc = s // 128          # k chunks
    qts = min(qts, s)      # q tile size (columns per MM1 matmul)
    nt = s // qts          # q tiles
    nqq = qts // 128       # 128-row output blocks per q tile

    nc = bacc.Bacc("TRN2", target_bir_lowering=False, debug=False,
                   num_devices=N_CORES)
    if mm1_dt == "bf16x2":
        qt_d = nc.declare_dram_parameter("qt", [hpc, 2, D, s], qk_dt,
                                         isOutput=False)
        kt_d = nc.declare_dram_parameter("kt", [hpc, 2, D, s], qk_dt,
                                         isOutput=False)
    else:
        qt_d = nc.declare_dram_parameter("qt", [hpc, D, s], qk_dt,
                                         isOutput=False)
        kt_d = nc.declare_dram_parameter("kt", [hpc, D, s], qk_dt,
                                         isOutput=False)
    v_d = nc.declare_dram_parameter("v", [hpc, 128, (s // 128) * 65], pv_dt,
                                    isOutput=False)
    nmt_d = nc.declare_dram_parameter("nmt", [s, s], pv_dt, isOutput=False)
    out_d = nc.declare_dram_parameter("out", [hpc, s, D], f32, isOutput=True)

    with tile.TileContext(nc) as tc:
        import contextlib
        with contextlib.ExitStack() as ctx:
            maskp = ctx.enter_context(tc.tile_pool(name="maskp", bufs=1))
            identp = ctx.enter_context(tc.tile_pool(name="identp", bufs=1))
            qkp = ctx.enter_context(tc.tile_pool(name="qkp", bufs=2))
            vp = ctx.enter_context(tc.tile_pool(name="vp", bufs=4))
            ptp = ctx.enter_context(tc.tile_pool(name="ptp", bufs=6))
            otsbp = ctx.enter_context(tc.tile_pool(name="otsbp", bufs=4))
            outp = ctx.enter_context(tc.tile_pool(name="outp", bufs=8))
            recipp = ctx.enter_context(tc.tile_pool(name="recipp", bufs=8))
            ps_s = ctx.enter_context(
                tc.tile_pool(name="ps_s", bufs=2, space="PSUM"))
            ps_ot = ctx.enter_context(
                tc.tile_pool(name="ps_ot", bufs=1, space="PSUM"))
            ps_tr = ctx.enter_context(
                tc.tile_pool(name="ps_tr", bufs=2, space="PSUM"))

            def load_pair(pair):
                h0, h1 = 2 * pair, 2 * pair + 1
                if mm1_dt == "bf16x2":
                    kt2 = []
                    qt2 = []
                    for part in (0, 1):
                        k_ = qkp.tile([128, s], qk_dt, tag=f"kt2_{part}")
                        q_ = qkp.tile([128, s], qk_dt, tag=f"qt2_{part}")
                        nc.sync.dma_start(out=k_[0:64, :],
                                          in_=kt_d[h0, part, :, :])
                        nc.sync.dma_start(out=k_[64:128, :],
                                          in_=kt_d[h1, part, :, :])
                        nc.sync.dma_start(out=q_[0:64, :],
                                          in_=qt_d[h0, part, :, :])
                        nc.sync.dma_start(out=q_[64:128, :],
                                          in_=qt_d[h1, part, :, :])
                        kt2.append(k_)
                        qt2.append(q_)
                else:
                    kt2 = qkp.tile([128, s], qk_dt, tag="kt2")
                    qt2 = qkp.tile([128, s], qk_dt, tag="qt2")
                    nc.sync.dma_start(out=kt2[0:64, :], in_=kt_d[h0, :, :])
                    nc.sync.dma_start(out=kt2[64:128, :], in_=kt_d[h1, :, :])
                    nc.sync.dma_start(out=qt2[0:64, :], in_=qt_d[h0, :, :])
                    nc.sync.dma_start(out=qt2[64:128, :], in_=qt_d[h1, :, :])
                vaug = []
                for h in (h0, h1):
                    va = vp.tile([128, kc * 65], pv_dt, tag="vaug")
                    nc.sync.dma_start(out=va, in_=v_d[h, :, :])
                    vaug.append(va)
                return kt2, qt2, vaug

            # Prefetch pair 0 inputs before the big mask load so the first
            # matmuls can start immediately.
            pair0 = load_pair(0)

            # (1 - mask)^T resident for the whole kernel; one tile per
            # k-chunk so consumers only depend on their own chunk's DMA.
            nmt_sb = []
            for c in range(kc):
                tl_ = maskp.tile([128, s], pv_dt, tag=f"nmt{c}")
                nc.sync.dma_start(out=tl_,
                                  in_=nmt_d[c * 128:(c + 1) * 128, :])
                nmt_sb.append(tl_)

            ident = identp.tile([128, 128], f32)
            make_identity(nc, ident)

            zbias = identp.tile([128, 1], f32)
            nc.vector.memset(zbias, 0.0)

            def finalize(ot_ps, h, t):
                # ot_ps: [65, qts] PSUM = [V^T P | sums]^T accumulated.
                ot_sb = otsbp.tile([65, qts], f32, tag="ot_sb")
                nc.vector.tensor_copy(ot_sb, ot_ps)
                for qq in range(nqq):
                    tr = ps_tr.tile([128, 65], f32, tag="tr")
                    nc.tensor.transpose(
                        tr, ot_sb[:, qq * 128:(qq + 1) * 128],
                        ident[0:65, 0:65])
                    recip = recipp.tile([128, 1], f32, tag="recip")
                    nc.vector.reciprocal(out=recip, in_=tr[:, 64:65])
                    out_t = outp.tile([128, D], f32, tag="out_t")
                    nc.vector.tensor_scalar_mul(out_t, tr[:, 0:64], recip)
                    q0 = t * qts + qq * 128
                    nc.sync.dma_start(out=out_d[h, q0:q0 + 128, :],
                                      in_=out_t)

            import contextlib as _cl
            loop_cm = tc.For_i(0, loop_reps, 1) if loop_reps else _cl.nullcontext()
            with loop_cm:
              for rep in range(reps):
                for pair in range(hpc // 2):
                    h0, h1 = 2 * pair, 2 * pair + 1
                    if rep == 0 and pair == 0 and not loop_reps:
                        kt2, qt2, vaug = pair0
                    else:
                        kt2, qt2, vaug = load_pair(pair)

                    for t in range(nt):
                      ot0 = ps_ot.tile([65, qts], f32, tag="ot0")
                      ot1 = ps_ot.tile([65, qts], f32, tag="ot1")
                      for c in range(kc):
                          ps = ps_s.tile([128, 2 * qts], f32, tag="ps")
                          # ST[k-chunk, q-tile] for both heads, row-packed.
                          if mm1_dt == "bf16x2":
                              # hi*hi + hi*lo + lo*hi accumulated -> ~fp32
                              # precision scores from bf16 hardware matmuls.
                              terms = ((0, 0), (0, 1), (1, 0))
                              for i, (kp, qp) in enumerate(terms):
                                  st = i == 0
                                  sp = i == len(terms) - 1
                                  nc.tensor.matmul(
                                      ps[:, 0:qts],
                                      kt2[kp][0:64, c * 128:(c + 1) * 128],
                                      qt2[qp][0:64, t * qts:(t + 1) * qts],
                                      start=st, stop=sp, tile_position=(0, 0))
                                  nc.tensor.matmul(
                                      ps[:, qts:2 * qts],
                                      kt2[kp][64:128, c * 128:(c + 1) * 128],
                                      qt2[qp][64:128, t * qts:(t + 1) * qts],
                                      start=st, stop=sp, tile_position=(64, 0))
                          else:
                              nc.tensor.matmul(
                                  ps[:, 0:qts],
                                  kt2[0:64, c * 128:(c + 1) * 128],
                                  qt2[0:64, t * qts:(t + 1) * qts],
                                  start=True, stop=True, tile_position=(0, 0))
                              nc.tensor.matmul(
                                  ps[:, qts:2 * qts],
                                  kt2[64:128, c * 128:(c + 1) * 128],
                                  qt2[64:128, t * qts:(t + 1) * qts],
                                  start=True, stop=True,
                                  tile_position=(64, 0))
                          pt = ptp.tile([128, 2 * qts], pv_dt, tag="pt")
                          nc.scalar.activation(out=pt, in_=ps, func=Exp,
                                               bias=zbias, scale=0.125)
                          nm = nmt_sb[c][:, t * qts:(t + 1) * qts]
                          # one DVE op covers both heads: the mask operand
                          # repeats via a stride-0 free dim.
                          nm2 = bass.AP(
                              tensor=nm.tensor, offset=nm.offset,
                              ap=[nm.ap[0], [0, 2], nm.ap[-1]])
                          nc.vector.tensor_mul(pt, pt, nm2)
                          nc.tensor.matmul(
                              ot0, vaug[0][:, c * 65:(c + 1) * 65],
                              pt[:, 0:qts],
                              start=(c == 0), stop=(c == kc - 1))
                          nc.tensor.matmul(
                              ot1, vaug[1][:, c * 65:(c + 1) * 65],
                              pt[:, qts:2 * qts],
                              start=(c == 0), stop=(c == kc - 1))
                      finalize(ot0, h0, t)
                      finalize(ot1, h1, t)

    nc.compile()
    return nc


def _to_bf16(x):
    return np.ascontiguousarray(x).astype(ml_dtypes.bfloat16)


def _to_f16(x):
    return np.ascontiguousarray(x).astype(np.float16)


MM1_DT = "float16"  # "float16"|"bfloat16"|"bf16x2"|"float32r"|"float32"


def kernel(Q, K, V, mask):
    """Full-input entry point: shards across 8 NeuronCores and gathers."""
    from concourse.bass_utils import run_bass_kernel_spmd

    Q = np.asarray(Q, dtype=np.float32)
    K = np.asarray(K, dtype=np.float32)
    V = np.asarray(V, dtype=np.float32)
    mask = np.asarray(mask)

    # Host-side layout prep (part of sharding): transpose Q/K to [D, S],
    # mask -> (1 - m)^T per batch, all in bf16.
    if MM1_DT in ("bfloat16", "float16"):
        cast = _to_bf16 if MM1_DT == "bfloat16" else _to_f16
        qt = cast(Q.transpose(0, 1, 3, 2))        # [B, H, D, S]
        kt = cast(K.transpose(0, 1, 3, 2))        # [B, H, D, S]
    elif MM1_DT == "bf16x2":
        def _split(x):
            xt = np.ascontiguousarray(x.transpose(0, 1, 3, 2))
            hi = xt.astype(ml_dtypes.bfloat16)
            lo = (xt - hi.astype(np.float32)).astype(ml_dtypes.bfloat16)
            return np.stack([hi, lo], axis=2)     # [B, H, 2, D, S]
        qt = _split(Q)
        kt = _split(K)
    else:
        qt = np.ascontiguousarray(Q.transpose(0, 1, 3, 2))
        kt = np.ascontiguousarray(K.transpose(0, 1, 3, 2))
    kc = S // 128
    vr = V.reshape(B, H, kc, 128, D)
    vaug = np.ones((B, H, kc, 128, D + 1), dtype=np.float32)
    vaug[..., :D] = vr
    # -> [B, H, 128, kc*(D+1)] exactly matching the SBUF tile layout
    vb = _to_f16(vaug.transpose(0, 1, 3, 2, 4).reshape(B, H, 128, kc * (D + 1)))
    nmt = _to_f16((~mask[:, 0]).transpose(0, 2, 1))  # [B, S, S]

    if MM1_DT not in _NC_CACHE:
        _NC_CACHE[MM1_DT] = build_attention_nc(mm1_dt=MM1_DT)
    nc = _NC_CACHE[MM1_DT]

    in_maps = []
    for c in range(N_CORES):
        b = c // 2
        hs = (c % 2) * HEADS_PER_CORE
        in_maps.append({
            "qt": np.ascontiguousarray(qt[b, hs:hs + HEADS_PER_CORE]),
            "kt": np.ascontiguousarray(kt[b, hs:hs + HEADS_PER_CORE]),
            "v": np.ascontiguousarray(vb[b, hs:hs + HEADS_PER_CORE]),
            "nmt": np.ascontiguousarray(nmt[b]),
        })

    res = None
    for attempt in range(3):
        try:
            res = run_bass_kernel_spmd(nc, in_maps, list(range(N_CORES)))
            break
        except Exception:
            if attempt == 2:
                raise
            import time
            time.sleep(2.0)

    out = np.empty((B, H, S, D), dtype=np.float32)
    for c in range(N_CORES):
        b = c // 2
        hs = (c % 2) * HEADS_PER_CORE
        out[b, hs:hs + HEADS_PER_CORE] = res.results[c]["out"]
    return out



# revision 3
# speedup vs baseline: 3.3391x; 3.3391x over previous
"""Masked attention kernel for Trainium2, sharded over 8 NeuronCores.

Problem: B=4, H=16, S=2048, D=64 attention with a boolean mask shared
across heads:  out = softmax((QK^T masked to -1e9) / sqrt(D)) @ V.

Sharding: batch*heads across 8 cores -> each core owns one batch element's
half of the heads (8 heads), so the [S, S] mask is loaded once per core.

The end-to-end wall time of kernel() is dominated by the axon host->device
tunnel (~50 MB/s), so the host ships the minimum number of bytes and ALL
layout preparation happens on-device:

  - Q, K, V are shipped as contiguous fp16 casts of the natural [S, D]
    layout (no host transposes).  The [D, S] operand layouts MM1 needs are
    produced on-device with XBAR DMA transposes of [128, 128] tiles (two
    heads' 64 columns packed side by side by the load DMA so the transpose
    yields the row-tiled (h,d) partition layout the PE matmuls want).
  - The mask is shipped bit-packed (packbits along k, little-endian), u16
    words: 0.5 MB/core instead of 8.4 MB fp16.  On-device: one XBAR
    transpose pass gives mbt[p, q] = bits for k in [16p, 16p+16), then 16
    DVE tensor_scalar ops ((x >> b) & 1 -> fp16) produce the 16 resident
    mask tiles directly.  k-chunk b therefore covers the strided set
    k = 16p + b; V and K rows are loaded with the same permutation by the
    DMA access pattern, which is legal because attention is order-invariant
    over the contraction index k.
  - The output is written fp16 (halves both the donated zero-buffer upload
    and the result download), cast back to fp32 on host.

Per-core compute strategy (per pair of heads), unchanged from the proven
baseline:
  - Scores are computed TRANSPOSED: ST[k, q] = K @ Q^T, so that the
    post-softmax matrix P^T[k, q] is already laid out with the contraction
    dim (k) on partitions for the second matmul.
  - D=64 contraction lets two heads share the 128x128 PE array via
    row tiling (tile_position (0,0) / (64,0)).
  - exp on the scalar engine directly from PSUM (scale=1/8 folded in),
    output in fp16; mask applied as a multiply by the unpacked (1-m) tile
    on the vector engine.  exp(-1e9/8) == 0 in fp32, so multiplying the
    exp by (1-m) is exactly equivalent to the reference's additive -1e9
    mask.
  - Second matmul uses V augmented with a ones column -> PSUM accumulator
    [65, 512] holds both the output numerator (rows 0..63, transposed) and
    the softmax denominators (row 64).
  - Finalize: PE transpose [65,128] -> [128,65], reciprocal of the sums
    column and a per-partition tensor_scalar multiply, then DMA out fp16.
"""

import numpy as np

B, H, S, D = 4, 16, 2048, 64
N_CORES = 8
HEADS_PER_CORE = (B * H) // N_CORES  # 8

_NC_CACHE = {}


def build_attention_nc(hpc=HEADS_PER_CORE, qts=512):
    """Build the per-core Bass program."""
    import contextlib

    import concourse.bass as bass
    import concourse.mybir as mybir
    import concourse.tile as tile
    from concourse import bacc
    from concourse.masks import make_identity

    f16 = mybir.dt.float16
    f32 = mybir.dt.float32
    u16 = mybir.dt.uint16
    Exp = mybir.ActivationFunctionType.Exp
    Rsh = mybir.AluOpType.logical_shift_right
    And = mybir.AluOpType.bitwise_and

    s = S
    kc = 16          # k chunks == bits per u16 mask word
    nt = s // qts    # q tiles
    nqq = qts // 128

    nc = bacc.Bacc("TRN2", target_bir_lowering=False, debug=False,
                   num_devices=N_CORES)
    q_d = nc.declare_dram_parameter("q", [hpc, s, D], f16, isOutput=False)
    k_d = nc.declare_dram_parameter("k", [hpc, s, D], f16, isOutput=False)
    v_d = nc.declare_dram_parameter("v", [hpc, s, D], f16, isOutput=False)
    mb_d = nc.declare_dram_parameter("mb", [s, 128], u16, isOutput=False)
    out_d = nc.declare_dram_parameter("out", [hpc, s, D], f16, isOutput=True)

    with tile.TileContext(nc) as tc:
        with contextlib.ExitStack() as ctx:
            maskp = ctx.enter_context(tc.tile_pool(name="maskp", bufs=1))
            identp = ctx.enter_context(tc.tile_pool(name="identp", bufs=1))
            qkp = ctx.enter_context(tc.tile_pool(name="qkp", bufs=2))
            vp = ctx.enter_context(tc.tile_pool(name="vp", bufs=4))
            ptp = ctx.enter_context(tc.tile_pool(name="ptp", bufs=6))
            otsbp = ctx.enter_context(tc.tile_pool(name="otsbp", bufs=4))
            outp = ctx.enter_context(tc.tile_pool(name="outp", bufs=8))
            recipp = ctx.enter_context(tc.tile_pool(name="recipp", bufs=8))
            ps_s = ctx.enter_context(
                tc.tile_pool(name="ps_s", bufs=2, space="PSUM"))
            ps_ot = ctx.enter_context(
                tc.tile_pool(name="ps_ot", bufs=1, space="PSUM"))
            ps_tr = ctx.enter_context(
                tc.tile_pool(name="ps_tr", bufs=2, space="PSUM"))

            def load_pair(pair):
                """DMA one pair of heads and build the transposed/augmented
                SBUF operands on-device."""
                h0, h1 = 2 * pair, 2 * pair + 1
                qin = qkp.tile([128, s], f16, tag="qin")
                kin = qkp.tile([128, s], f16, tag="kin")
                qin4 = qin.rearrange("p (c h j) -> p c h j", c=kc, h=2)
                kin4 = kin.rearrange("p (c h j) -> p c h j", c=kc, h=2)
                for i, h in enumerate((h0, h1)):
                    # Q chunks in natural q order: chunk c partition p = q
                    # index c*128+p.
                    nc.sync.dma_start(
                        out=qin4[:, :, i, :],
                        in_=q_d[h].rearrange("(c p) j -> p c j", c=kc))
                    # K rows bit-permuted to match the mask unpack: chunk c
                    # partition p = k index 16p+c.
                    nc.sync.dma_start(
                        out=kin4[:, :, i, :],
                        in_=k_d[h].rearrange("(p c) j -> p c j", p=128))
                # XBAR-transpose each [128 (s), 128 (h,d)] block into the
                # [128 (h,d), s] matmul operand layout.
                qt2 = qkp.tile([128, s], f16, tag="qt2")
                kt2 = qkp.tile([128, s], f16, tag="kt2")
                for c in range(kc):
                    blk = slice(c * 128, (c + 1) * 128)
                    nc.sync.dma_start_transpose(qt2[:, blk], qin[:, blk])
                    nc.sync.dma_start_transpose(kt2[:, blk], kin[:, blk])
                # V with the same k permutation, plus the ones column that
                # accumulates the softmax denominators in MM2.
                vaug = []
                for h in (h0, h1):
                    va = vp.tile([128, kc * 65], f16, tag="vaug")
                    va3 = va.rearrange("p (c j) -> p c j", c=kc)
                    nc.vector.memset(va3[:, :, 64:65], 1.0)
                    nc.sync.dma_start(
                        out=va3[:, :, 0:64],
                        in_=v_d[h].rearrange("(p c) j -> p c j", p=128))
                    vaug.append(va)
                return kt2, qt2, vaug

            # Prefetch pair 0 inputs before the mask prologue so the first
            # matmuls can start immediately.
            pair0 = load_pair(0)

            # Mask prologue: load packed bits, XBAR-transpose to put k on
            # partitions, then unpack each bit b into a resident fp16 tile
            # nmt_sb[b][p, q] = 1 - mask[q, 16p+b].
            mbits = maskp.tile([128, kc * 128], u16, tag="mbits")
            nc.sync.dma_start(
                out=mbits.rearrange("p (c j) -> p c j", c=kc),
                in_=mb_d.rearrange("(c p) j -> p c j", c=kc))
            mbt = maskp.tile([128, s], u16, tag="mbt")
            for c in range(kc):
                blk = slice(c * 128, (c + 1) * 128)
                nc.sync.dma_start_transpose(mbt[:, blk], mbits[:, blk])
            # The bitVec tensor_scalar can't cast u16->f16 in one op, so
            # shift+and to a u16 temp, then a casting multiply-by-1.
            nmt_sb = []
            for b in range(kc):
                tmp = maskp.tile([128, s], u16, tag=f"mtmp{b % 2}")
                nc.vector.tensor_scalar(tmp, mbt, b, 1, Rsh, And)
                tl_ = maskp.tile([128, s], f16, tag=f"nmt{b}")
                nc.vector.tensor_scalar(tl_, tmp, 1.0, None,
                                        mybir.AluOpType.mult)
                nmt_sb.append(tl_)

            ident = identp.tile([128, 128], f32)
            make_identity(nc, ident)

            zbias = identp.tile([128, 1], f32)
            nc.vector.memset(zbias, 0.0)

            def finalize(ot_ps, h, t):
                # ot_ps: [65, qts] PSUM = [V^T P | sums]^T accumulated.
                ot_sb = otsbp.tile([65, qts], f32, tag="ot_sb")
                nc.vector.tensor_copy(ot_sb, ot_ps)
                for qq in range(nqq):
                    tr = ps_tr.tile([128, 65], f32, tag="tr")
                    nc.tensor.transpose(
                        tr, ot_sb[:, qq * 128:(qq + 1) * 128],
                        ident[0:65, 0:65])
                    recip = recipp.tile([128, 1], f32, tag="recip")
                    nc.vector.reciprocal(out=recip, in_=tr[:, 64:65])
                    out_t = outp.tile([128, D], f16, tag="out_t")
                    nc.vector.tensor_scalar_mul(out_t, tr[:, 0:64], recip)
                    q0 = t * qts + qq * 128
                    nc.sync.dma_start(out=out_d[h, q0:q0 + 128, :],
                                      in_=out_t)

            for pair in range(hpc // 2):
                h0, h1 = 2 * pair, 2 * pair + 1
                if pair == 0:
                    kt2, qt2, vaug = pair0
                else:
                    kt2, qt2, vaug = load_pair(pair)

                for t in range(nt):
                    ot0 = ps_ot.tile([65, qts], f32, tag="ot0")
                    ot1 = ps_ot.tile([65, qts], f32, tag="ot1")
                    for c in range(kc):
                        ps = ps_s.tile([128, 2 * qts], f32, tag="ps")
                        # ST[k-chunk, q-tile] for both heads, row-packed.
                        nc.tensor.matmul(
                            ps[:, 0:qts],
                            kt2[0:64, c * 128:(c + 1) * 128],
                            qt2[0:64, t * qts:(t + 1) * qts],
                            start=True, stop=True, tile_position=(0, 0))
                        nc.tensor.matmul(
                            ps[:, qts:2 * qts],
                            kt2[64:128, c * 128:(c + 1) * 128],
                            qt2[64:128, t * qts:(t + 1) * qts],
                            start=True, stop=True, tile_position=(64, 0))
                        pt = ptp.tile([128, 2 * qts], f16, tag="pt")
                        nc.scalar.activation(out=pt, in_=ps, func=Exp,
                                             bias=zbias, scale=0.125)
                        nm = nmt_sb[c][:, t * qts:(t + 1) * qts]
                        # one DVE op covers both heads: the mask operand
                        # repeats via a stride-0 free dim.
                        nm2 = bass.AP(
                            tensor=nm.tensor, offset=nm.offset,
                            ap=[nm.ap[0], [0, 2], nm.ap[-1]])
                        nc.vector.tensor_mul(pt, pt, nm2)
                        nc.tensor.matmul(
                            ot0, vaug[0][:, c * 65:(c + 1) * 65],
                            pt[:, 0:qts],
                            start=(c == 0), stop=(c == kc - 1))
                        nc.tensor.matmul(
                            ot1, vaug[1][:, c * 65:(c + 1) * 65],
                            pt[:, qts:2 * qts],
                            start=(c == 0), stop=(c == kc - 1))
                    finalize(ot0, h0, t)
                    finalize(ot1, h1, t)

    nc.compile()
    return nc


def kernel(Q, K, V, mask):
    """Full-input entry point: shards across 8 NeuronCores and gathers."""
    from concourse.bass_utils import run_bass_kernel_spmd

    Q = np.asarray(Q)
    K = np.asarray(K)
    V = np.asarray(V)
    mask = np.asarray(mask)

    # Host-side prep: contiguous fp16 casts only; all transposes and the
    # mask unpack happen on-device.
    qh = Q.astype(np.float16)
    kh = K.astype(np.float16)
    vh = V.astype(np.float16)
    mb = np.packbits(~mask[:, 0], axis=-1,
                     bitorder="little").view(np.uint16)  # [B, S, S/16]

    if "nc" not in _NC_CACHE:
        _NC_CACHE["nc"] = build_attention_nc()
    nc = _NC_CACHE["nc"]

    in_maps = []
    for c in range(N_CORES):
        b = c // 2
        hs = (c % 2) * HEADS_PER_CORE
        in_maps.append({
            "q": qh[b, hs:hs + HEADS_PER_CORE],
            "k": kh[b, hs:hs + HEADS_PER_CORE],
            "v": vh[b, hs:hs + HEADS_PER_CORE],
            "mb": mb[b],
        })

    res = None
    for attempt in range(3):
        try:
            res = run_bass_kernel_spmd(nc, in_maps, list(range(N_CORES)))
            break
        except Exception:
            if attempt == 2:
                raise
            import time
            time.sleep(2.0)

    out = np.empty((B, H, S, D), dtype=np.float32)
    for c in range(N_CORES):
        b = c // 2
        hs = (c % 2) * HEADS_PER_CORE
        out[b, hs:hs + HEADS_PER_CORE] = res.results[c]["out"]
    return out
